# revision 17
# baseline (speedup 1.0000x reference)
"""Multi-head self-attention (dense_transformer) on 8 Trainium2 NeuronCores.

Problem: B=4, S=2048, D=1024, H=16, E=1024, HD=64.
reference returns (output [B,S,D] f32, weights [B,H,S,S] f32).

Sharding: batch x head-group. Core c handles batch b = c//2 and heads
hs = (c%2)*8 .. hs+8 (8 heads). Each core:
  phase 1: QKV projection from x[b] (fp16 operands, fp32 psum).
           qT,kT stored [head-pair d-dims (128p), S]; V stored [S(k), ch]
           augmented with a ones column per head (for softmax sums).
  phase 2: per head, scores in BOTH orientations on PE:
           sA [q,k] -> ACT exp (+row-sum accum) -> DVE 1/sum scale -> DMA out
           sB [k,q] -> ACT exp -> fp16 -> PE [V|1]^T @ expB = psum [d|sum, q]
           -> DVE recip row -> PE K=1 broadcast matmul -> DVE mult -> outT.
  phase 3: out-proj partial from outT [ch, row], DMA.
Host: shards/transposes/casts inputs, sums the two partial outputs per batch.
"""

import sys

import numpy as np

if "/opt/trn_rl_repo" not in sys.path:
    sys.path.insert(0, "/opt/trn_rl_repo")

B, S, D = 4, 2048, 1024
H = 16
E = 1024
HD = 64
SCALING = float(HD) ** -0.5

P = 128
NHC = 8          # heads per core
CH = NHC * HD    # 512 channels per core
N_CORES = 8
VW = HD + 1      # v columns per head incl. ones column

_NC_CACHE = None


def _build_nc():
    import concourse.bacc as bacc
    import concourse.mybir as mybir
    import concourse.tile as tile

    fp16 = mybir.dt.float16
    f32 = mybir.dt.float32
    Exp = mybir.ActivationFunctionType.Exp
    Mult = mybir.AluOpType.mult

    nc = bacc.Bacc("TRN2", target_bir_lowering=False, debug=False)

    xt = nc.dram_tensor("xt", [D, S], fp16, kind="ExternalInput")
    wqt = nc.dram_tensor("wqt", [D, CH], fp16, kind="ExternalInput")
    wkt = nc.dram_tensor("wkt", [D, CH], fp16, kind="ExternalInput")
    wvt = nc.dram_tensor("wvt", [D, CH], fp16, kind="ExternalInput")
    wot = nc.dram_tensor("wot", [CH, D], fp16, kind="ExternalInput")
    bq = nc.dram_tensor("bq", [P, CH // P], f32, kind="ExternalInput")
    bk = nc.dram_tensor("bk", [P, CH // P], f32, kind="ExternalInput")
    bvb = nc.dram_tensor("bvb", [P, CH], f32, kind="ExternalInput")
    wout = nc.dram_tensor("wout", [NHC, S, S], f32, kind="ExternalOutput")
    pout = nc.dram_tensor("pout", [S, D], f32, kind="ExternalOutput")

    FT = D // P          # 8 feature tiles
    RT = S // P          # 16 row tiles
    RG = S // 512        # 4 row/col groups of 512
    CT = CH // P         # 4 chan tiles == head pairs
    KC = S // P          # 16 key chunks
    QG = S // 512        # 4 query groups

    with tile.TileContext(nc) as tc:
        with tc.tile_pool(name="persist", bufs=1) as persist:
            qt_sb = persist.tile([P, CT, S], fp16, tag="qt")
            kt_sb = persist.tile([P, CT, S], fp16, tag="kt")
            vaug_sb = persist.tile([P, RT, NHC * VW], fp16, tag="vaug")
            outT_sb = persist.tile([P, CT, S], fp16, tag="outT")
            wot_sb = persist.tile([P, CT, D], fp16, tag="wot")
            bq_sb = persist.tile([P, CT], f32, tag="bq")
            bk_sb = persist.tile([P, CT], f32, tag="bk")
            bvb_sb = persist.tile([P, CH], f32, tag="bvb")
            ones1 = persist.tile([1, 64], f32, tag="ones1")

            nc.gpsimd.memset(vaug_sb[:], 1.0)
            nc.gpsimd.memset(ones1[:], 1.0)
            nc.sync.dma_start(wot_sb[:], wot.ap().rearrange("(ct p) d -> p ct d", p=P))
            nc.sync.dma_start(bq_sb[:], bq.ap())
            nc.sync.dma_start(bk_sb[:], bk.ap())
            nc.sync.dma_start(bvb_sb[:], bvb.ap())

            # psum pools shared by all phases (8 banks total)
            psA_cm = tc.tile_pool(name="psA", bufs=1, space="PSUM")
            psB_cm = tc.tile_pool(name="psB", bufs=1, space="PSUM")
            psA = psA_cm.__enter__()
            psB = psB_cm.__enter__()

            # ---------------- phase 1: QKV ----------------
            with (
                tc.tile_pool(name="ph1", bufs=1) as ph1,
            ):
                xt_sb = ph1.tile([P, FT, S], fp16, tag="xt")
                wq_sb = ph1.tile([P, FT, CH], fp16, tag="wq")
                wk_sb = ph1.tile([P, FT, CH], fp16, tag="wk")
                wv_sb = ph1.tile([P, FT, CH], fp16, tag="wv")
                nc.sync.dma_start(xt_sb[:], xt.ap().rearrange("(ft p) r -> p ft r", p=P))
                nc.sync.dma_start(wq_sb[:], wqt.ap().rearrange("(ft p) c -> p ft c", p=P))
                nc.sync.dma_start(wk_sb[:], wkt.ap().rearrange("(ft p) c -> p ft c", p=P))
                nc.sync.dma_start(wv_sb[:], wvt.ap().rearrange("(ft p) c -> p ft c", p=P))

                # q,k in [chan, row] orientation
                for dst, wsb, bias in ((qt_sb, wq_sb, bq_sb), (kt_sb, wk_sb, bk_sb)):
                    for ct in range(CT):
                        for rgp in range(RG // 2):
                            ps = psB.tile([P, 1024], f32, tag="psb")
                            for sub in range(2):
                                rg = 2 * rgp + sub
                                for ft in range(FT):
                                    nc.tensor.matmul(
                                        ps[:, sub * 512:(sub + 1) * 512],
                                        wsb[:, ft, ct * P:(ct + 1) * P],
                                        xt_sb[:, ft, rg * 512:(rg + 1) * 512],
                                        start=(ft == 0), stop=(ft == FT - 1),
                                    )
                            nc.vector.tensor_scalar_add(
                                dst[:, ct, rgp * 1024:(rgp + 1) * 1024], ps[:],
                                bias[:, ct:ct + 1],
                            )
                # v in [row, chan] orientation, interleaved into vaug
                for rtg in range(RT // 4):
                    ps = psA.tile([P, S], f32, tag="psa")
                    for sub in range(4):
                        rt = 4 * rtg + sub
                        cs = sub * 512
                        for ft in range(FT):
                            nc.tensor.matmul(
                                ps[:, cs:cs + 512],
                                xt_sb[:, ft, rt * P:(rt + 1) * P],
                                wv_sb[:, ft, :],
                                start=(ft == 0), stop=(ft == FT - 1),
                            )
                        nc.vector.tensor_tensor(
                            vaug_sb[:, rt].rearrange("p (h c) -> p h c", h=NHC)[:, :, :HD],
                            ps[:, cs:cs + 512].rearrange("p (h c) -> p h c", h=NHC),
                            bvb_sb[:].rearrange("p (h c) -> p h c", h=NHC),
                            mybir.AluOpType.add,
                        )

            # ---------------- phase 2: attention ----------------
            with (
                tc.tile_pool(name="expa", bufs=6) as expa_pool,
                tc.tile_pool(name="expb", bufs=18) as expb_pool,
                tc.tile_pool(name="small", bufs=24) as small_pool,
                tc.tile_pool(name="med", bufs=4) as med_pool,
            ):
                def emit_b_unit(p, qg, kc, expb_tiles):
                    psb = psB.tile([P, 1024], f32, tag="psb")
                    for half in (0, 1):
                        po = 64 * half
                        nc.tensor.matmul(
                            psb[:, half * 512:(half + 1) * 512],
                            kt_sb[po:po + 64, p, kc * P:(kc + 1) * P],
                            qt_sb[po:po + 64, p, qg * 512:(qg + 1) * 512],
                            start=True, stop=True,
                        )
                    ebt = expb_pool.tile([P, 1024], fp16, tag="ebt")
                    nc.scalar.activation(ebt[:], psb[:], Exp)
                    expb_tiles.append(ebt)

                def emit_a_unit(p, qg, j, half):
                    h = 2 * p + half
                    po = 64 * half
                    qt = qg * 4 + j
                    psa = psA.tile([P, S], f32, tag="psa")
                    for kg in range(RG):
                        nc.tensor.matmul(
                            psa[:, kg * 512:(kg + 1) * 512],
                            qt_sb[po:po + 64, p, qt * P:(qt + 1) * P],
                            kt_sb[po:po + 64, p, kg * 512:(kg + 1) * 512],
                            start=True, stop=True,
                        )
                    eat = expa_pool.tile([P, S], f32, tag="eat")
                    sums = small_pool.tile([P, 1], f32, tag="sums")
                    nc.scalar.activation(eat[:], psa[:], Exp, accum_out=sums[:, 0:1])
                    recip = small_pool.tile([P, 1], f32, tag="recip")
                    nc.vector.reciprocal(recip[:], sums[:])
                    nc.vector.tensor_scalar_mul(eat[:], eat[:], recip[:, 0:1])
                    nc.sync.dma_start(wout.ap()[h, qt * P:(qt + 1) * P, :], eat[:])

                def emit_av_unit(p, qg, expb_tiles):
                    # both heads of the pair share one [128, 1024] psum tile
                    # borrowed from the psB pool (same tag -> same slots)
                    pav = psB.tile([P, 1024], f32, tag="psb")
                    for half in (0, 1):
                        h = 2 * p + half
                        cs = half * 512
                        for kc in range(KC):
                            nc.tensor.matmul(
                                pav[0:VW, cs:cs + 512],
                                vaug_sb[:, kc, h * VW:(h + 1) * VW],
                                expb_tiles[kc][:, cs:cs + 512],
                                start=(kc == 0), stop=(kc == KC - 1),
                            )
                    for half in (0, 1):
                        po = 64 * half
                        cs = half * 512
                        rrow = med_pool.tile([1, 512], f32, tag="rrow")
                        nc.vector.reciprocal(rrow[:], pav[HD:VW, cs:cs + 512])
                        nc.tensor.matmul(
                            pav[64:128, cs:cs + 512], ones1[:], rrow[:],
                            start=True, stop=True,
                        )
                        rb_sb = med_pool.tile([64, 512], f32, tag="rb")
                        nc.vector.tensor_copy(rb_sb[:], pav[64:128, cs:cs + 512])
                        nc.vector.tensor_tensor(
                            outT_sb[po:po + 64, p, qg * 512:(qg + 1) * 512],
                            pav[0:HD, cs:cs + 512], rb_sb[:], Mult,
                        )

                for p in range(CT):          # head pair p -> heads 2p, 2p+1
                    for qg in range(QG):
                        expb_tiles = []
                        # interleave B and A units (2:1) so ACT always has work
                        for i in range(8):
                            emit_b_unit(p, qg, 2 * i, expb_tiles)
                            emit_b_unit(p, qg, 2 * i + 1, expb_tiles)
                            emit_a_unit(p, qg, i % 4, i // 4)
                        emit_av_unit(p, qg, expb_tiles)

            # ---------------- phase 3: output projection ----------------
            psB_cm.__exit__(None, None, None)
            psA_cm.__exit__(None, None, None)
            with (
                tc.tile_pool(name="ph3", bufs=1) as ph3,
                tc.tile_pool(name="psO", bufs=2, space="PSUM") as psO,
            ):
                pout_sb = ph3.tile([P, RT, D], f32, tag="pout")
                for rt in range(RT):
                    for dg in range(2):
                        pso = psO.tile([P, 512], f32, tag="pso")
                        for c in range(CT):
                            nc.tensor.matmul(
                                pso[:],
                                outT_sb[:, c, rt * P:(rt + 1) * P],
                                wot_sb[:, c, dg * 512:(dg + 1) * 512],
                                start=(c == 0), stop=(c == CT - 1),
                            )
                        nc.vector.tensor_copy(
                            pout_sb[:, rt, dg * 512:(dg + 1) * 512], pso[:]
                        )
                nc.sync.dma_start(
                    pout.ap().rearrange("(rt p) d -> p rt d", p=P), pout_sb[:]
                )

    nc.compile()
    return nc


def _get_nc():
    global _NC_CACHE
    if _NC_CACHE is None:
        _NC_CACHE = _build_nc()
    return _NC_CACHE


def _prep_core_inputs(c, x, w_in, b_in, w_out):
    b = c // 2
    hs = (c % 2) * NHC
    ch = slice(hs * HD, hs * HD + CH)

    wq = (w_in[0 * E:1 * E][ch] * SCALING).astype(np.float16)
    wk = w_in[1 * E:2 * E][ch].astype(np.float16)
    wv = w_in[2 * E:3 * E][ch].astype(np.float16)
    bq_ = (b_in[0 * E:1 * E][ch] * SCALING).astype(np.float32)
    bk_ = b_in[1 * E:2 * E][ch].astype(np.float32)
    bv_ = b_in[2 * E:3 * E][ch].astype(np.float32)

    return {
        "xt": np.ascontiguousarray(x[b].T).astype(np.float16),
        "wqt": np.ascontiguousarray(wq.T),
        "wkt": np.ascontiguousarray(wk.T),
        "wvt": np.ascontiguousarray(wv.T),
        "wot": np.ascontiguousarray(w_out[:, ch].T).astype(np.float16),
        "bq": np.ascontiguousarray(bq_.reshape(CH // P, P).T),
        "bk": np.ascontiguousarray(bk_.reshape(CH // P, P).T),
        "bvb": np.ascontiguousarray(
            np.broadcast_to(bv_, (P, CH)).astype(np.float32)),
    }


def run_sharded(x, w_in, b_in, w_out, b_out, trace=False, trace_cores=None):
    """Returns ((output, weights), BassKernelResults)."""
    from concourse import bass_utils

    nc = _get_nc()
    in_maps = [_prep_core_inputs(c, x, w_in, b_in, w_out) for c in range(N_CORES)]
    res = bass_utils.run_bass_kernel_spmd(
        nc, in_maps, core_ids=list(range(N_CORES)),
        trace=trace, trace_cores=trace_cores,
    )
    weights = np.empty((B, H, S, S), np.float32)
    output = np.empty((B, S, D), np.float32)
    for c in range(N_CORES):
        b = c // 2
        hs = (c % 2) * NHC
        weights[b, hs:hs + NHC] = res.results[c]["wout"]
    for b in range(B):
        output[b] = (res.results[2 * b]["pout"] + res.results[2 * b + 1]["pout"]
                     + b_out[None, :])
    return (output, weights), res


def kernel(x, w_in, b_in, w_out, b_out):
    x = np.asarray(x, dtype=np.float32)
    w_in = np.asarray(w_in, dtype=np.float32)
    b_in = np.asarray(b_in, dtype=np.float32)
    w_out = np.asarray(w_out, dtype=np.float32)
    b_out = np.asarray(b_out, dtype=np.float32)
    (output, weights), _ = run_sharded(x, w_in, b_in, w_out, b_out)
    return output, weights


# revision 21
# speedup vs baseline: 1.3558x; 1.3558x over previous
"""Multi-head self-attention (dense_transformer) on 8 Trainium2 NeuronCores.

Problem: B=4, S=2048, D=1024, H=16, E=1024, HD=64.
reference returns (output [B,S,D] f32, weights [B,H,S,S] f32).

Sharding: batch x head-group. Core c handles batch b = c//2 and heads
hs = (c%2)*8 .. hs+8 (8 heads). Per core:
  phase 1: QKV projection from x[b] (fp16 operands, fp32 psum).
           qT,kT stored [head-pair d-dims (128p), S]; V stored [S(k), ch]
           augmented with a ones column per head (softmax sums for free).
  phase 2: scores computed ONLY in [k, q] orientation (PE, head pairs
           row-packed), ACT exp -> fp16 expB tiles.
           attn*V: [V|1]^T @ expB -> psum [d|sum, q]; 1/sum row (DVE) is
           broadcast across partitions by a K=1 ones matmul (PE), then
           DVE multiply -> outT [ch, row] fp16.
           weights output: PE-transposes expB chunks into an fp16 psum
           tile [q, k-block]; per-q 1/sum (tiny PE transposes of the
           recip row) scales it during the psum->SBUF f32 copy
           (alternating DVE / ACT), then DMA.
  phase 3: out-proj partial from outT [ch, row], DMA.
Host: shards/transposes/casts inputs, sums the two partial outputs per batch.
"""

import sys

import numpy as np

if "/opt/trn_rl_repo" not in sys.path:
    sys.path.insert(0, "/opt/trn_rl_repo")

B, S, D = 4, 2048, 1024
H = 16
E = 1024
HD = 64
SCALING = float(HD) ** -0.5

P = 128
NHC = 8          # heads per core
CH = NHC * HD    # 512 channels per core
N_CORES = 8
VW = HD + 1      # v columns per head incl. ones column

_NC_CACHE = None


def _build_nc():
    import concourse.bacc as bacc
    import concourse.mybir as mybir
    import concourse.tile as tile
    from concourse.masks import make_identity

    fp16 = mybir.dt.float16
    f32 = mybir.dt.float32
    Exp = mybir.ActivationFunctionType.Exp
    Copy = mybir.ActivationFunctionType.Copy
    Mult = mybir.AluOpType.mult

    nc = bacc.Bacc("TRN2", target_bir_lowering=False, debug=False)

    xt = nc.dram_tensor("xt", [D, S], fp16, kind="ExternalInput")
    wqt = nc.dram_tensor("wqt", [D, CH], fp16, kind="ExternalInput")
    wkt = nc.dram_tensor("wkt", [D, CH], fp16, kind="ExternalInput")
    wvt = nc.dram_tensor("wvt", [D, CH], fp16, kind="ExternalInput")
    wot = nc.dram_tensor("wot", [CH, D], fp16, kind="ExternalInput")
    bq = nc.dram_tensor("bq", [P, CH // P], f32, kind="ExternalInput")
    bk = nc.dram_tensor("bk", [P, CH // P], f32, kind="ExternalInput")
    bvb = nc.dram_tensor("bvb", [P, CH], f32, kind="ExternalInput")
    wout = nc.dram_tensor("wout", [NHC, S, S], f32, kind="ExternalOutput")
    pout = nc.dram_tensor("pout", [S, D], f32, kind="ExternalOutput")

    FT = D // P          # 8 feature tiles
    RT = S // P          # 16 row tiles
    RG = S // 512        # 4 row/col groups of 512
    CT = CH // P         # 4 chan tiles == head pairs
    KC = S // P          # 16 key chunks
    QG = S // 512        # 4 query groups

    with tile.TileContext(nc) as tc:
        with tc.tile_pool(name="persist", bufs=1) as persist:
            qt_sb = persist.tile([P, CT, S], fp16, tag="qt")
            kt_sb = persist.tile([P, CT, S], fp16, tag="kt")
            vaug_sb = persist.tile([P, RT, NHC * VW], fp16, tag="vaug")
            outT_sb = persist.tile([P, CT, S], fp16, tag="outT")
            wot_sb = persist.tile([P, CT, D], fp16, tag="wot")
            bq_sb = persist.tile([P, CT], f32, tag="bq")
            bk_sb = persist.tile([P, CT], f32, tag="bk")
            bvb_sb = persist.tile([P, CH], f32, tag="bvb")
            ones1 = persist.tile([1, 64], f32, tag="ones1")
            idq = persist.tile([P, P], fp16, tag="idq")

            nc.gpsimd.memset(vaug_sb[:], 1.0)
            nc.gpsimd.memset(ones1[:], 1.0)
            make_identity(nc, idq[:])
            nc.sync.dma_start(wot_sb[:], wot.ap().rearrange("(ct p) d -> p ct d", p=P))
            nc.sync.dma_start(bq_sb[:], bq.ap())
            nc.sync.dma_start(bk_sb[:], bk.ap())
            nc.sync.dma_start(bvb_sb[:], bvb.ap())

            # psum pools shared by all phases:
            # psB: [128,1024] f32 slots x2 (4 banks) - scores / attn*V / recips
            # psT: [128,2048] fp16 slots x2 (4 banks) - transposed weights
            psB_cm = tc.tile_pool(name="psB", bufs=2, space="PSUM")
            psT_cm = tc.tile_pool(name="psT", bufs=2, space="PSUM")
            psB = psB_cm.__enter__()
            psT = psT_cm.__enter__()

            # ---------------- phase 1: QKV ----------------
            with tc.tile_pool(name="ph1", bufs=1) as ph1:
                xt_sb = ph1.tile([P, FT, S], fp16, tag="xt")
                wq_sb = ph1.tile([P, FT, CH], fp16, tag="wq")
                wk_sb = ph1.tile([P, FT, CH], fp16, tag="wk")
                wv_sb = ph1.tile([P, FT, CH], fp16, tag="wv")
                nc.sync.dma_start(xt_sb[:], xt.ap().rearrange("(ft p) r -> p ft r", p=P))
                nc.sync.dma_start(wq_sb[:], wqt.ap().rearrange("(ft p) c -> p ft c", p=P))
                nc.sync.dma_start(wk_sb[:], wkt.ap().rearrange("(ft p) c -> p ft c", p=P))
                nc.sync.dma_start(wv_sb[:], wvt.ap().rearrange("(ft p) c -> p ft c", p=P))

                # q,k in [chan, row] orientation
                for dst, wsb, bias in ((qt_sb, wq_sb, bq_sb), (kt_sb, wk_sb, bk_sb)):
                    for ct in range(CT):
                        for rgp in range(RG // 2):
                            ps = psB.tile([P, 1024], f32, tag="psb")
                            for sub in range(2):
                                rg = 2 * rgp + sub
                                for ft in range(FT):
                                    nc.tensor.matmul(
                                        ps[:, sub * 512:(sub + 1) * 512],
                                        wsb[:, ft, ct * P:(ct + 1) * P],
                                        xt_sb[:, ft, rg * 512:(rg + 1) * 512],
                                        start=(ft == 0), stop=(ft == FT - 1),
                                    )
                            nc.vector.tensor_scalar_add(
                                dst[:, ct, rgp * 1024:(rgp + 1) * 1024], ps[:],
                                bias[:, ct:ct + 1],
                            )
                # v in [row, chan] orientation, interleaved into vaug
                for rtg in range(RT // 2):
                    ps = psT.tile([P, 1024], f32, tag="pst")
                    for sub in range(2):
                        rt = 2 * rtg + sub
                        cs = sub * 512
                        for ft in range(FT):
                            nc.tensor.matmul(
                                ps[:, cs:cs + 512],
                                xt_sb[:, ft, rt * P:(rt + 1) * P],
                                wv_sb[:, ft, :],
                                start=(ft == 0), stop=(ft == FT - 1),
                            )
                        nc.vector.tensor_tensor(
                            vaug_sb[:, rt].rearrange("p (h c) -> p h c", h=NHC)[:, :, :HD],
                            ps[:, cs:cs + 512].rearrange("p (h c) -> p h c", h=NHC),
                            bvb_sb[:].rearrange("p (h c) -> p h c", h=NHC),
                            mybir.AluOpType.add,
                        )

            # ---------------- phase 2: attention ----------------
            with (
                tc.tile_pool(name="eat", bufs=5) as eat_pool,
                tc.tile_pool(name="expb", bufs=18) as expb_pool,
                tc.tile_pool(name="small", bufs=8) as small_pool,
                tc.tile_pool(name="med", bufs=4) as med_pool,
            ):
                def emit_b_unit(p, qg, kc, expb_tiles):
                    psb = psB.tile([P, 1024], f32, tag="psb")
                    for half in (0, 1):
                        po = 64 * half
                        nc.tensor.matmul(
                            psb[:, half * 512:(half + 1) * 512],
                            kt_sb[po:po + 64, p, kc * P:(kc + 1) * P],
                            qt_sb[po:po + 64, p, qg * 512:(qg + 1) * 512],
                            start=True, stop=True,
                        )
                    ebt = expb_pool.tile([P, 1024], fp16, tag="ebt")
                    nc.scalar.activation(ebt[:], psb[:], Exp)
                    expb_tiles.append(ebt)

                def emit_av_unit(p, qg, expb_tiles):
                    # both heads of the pair share one [128, 1024] psum tile
                    # borrowed from the psB pool. Rows 0:64 attn*V, row 64
                    # sums, rows 64:128 reciprocal broadcast (overwrites the
                    # sums row after it has been read).
                    pav = psB.tile([P, 1024], f32, tag="psb")
                    for half in (0, 1):
                        h = 2 * p + half
                        cs = half * 512
                        for kc in range(KC):
                            nc.tensor.matmul(
                                pav[0:VW, cs:cs + 512],
                                vaug_sb[:, kc, h * VW:(h + 1) * VW],
                                expb_tiles[kc][:, cs:cs + 512],
                                start=(kc == 0), stop=(kc == KC - 1),
                            )
                    rrows = []
                    for half in (0, 1):
                        cs = half * 512
                        rrow = med_pool.tile([1, 512], f32, tag="rrow")
                        nc.vector.reciprocal(rrow[:], pav[HD:VW, cs:cs + 512])
                        rrows.append(rrow)
                    for half in (0, 1):
                        po = 64 * half
                        cs = half * 512
                        nc.tensor.matmul(
                            pav[64:128, cs:cs + 512], ones1[:], rrows[half][:],
                            start=True, stop=True,
                        )
                        rb_sb = med_pool.tile([64, 512], f32, tag="rb")
                        nc.vector.tensor_copy(rb_sb[:], pav[64:128, cs:cs + 512])
                        nc.vector.tensor_tensor(
                            outT_sb[po:po + 64, p, qg * 512:(qg + 1) * 512],
                            pav[0:HD, cs:cs + 512], rb_sb[:], Mult,
                        )
                    # transpose the recip rows into per-q columns:
                    # recip_ps[:, half*4 + j] = rrows[half][0, j*128:(j+1)*128]^T
                    recip_ps = psB.tile([P, 8], f32, tag="psb")
                    for half in (0, 1):
                        for j in range(4):
                            nc.tensor.transpose(
                                recip_ps[:, half * 4 + j:half * 4 + j + 1],
                                rrows[half][:, j * P:(j + 1) * P],
                                ones1[0:1, 0:1],
                            )
                    recip_sb = small_pool.tile([P, 8], f32, tag="recip_sb")
                    nc.vector.tensor_copy(recip_sb[:], recip_ps[:])
                    return recip_sb

                def emit_w_unit(p, qg, j, half, expb_tiles, recip_sb, on_act):
                    """weights block (h, qt): transpose 16 expB chunks into an
                    fp16 psum tile, scale by per-q recip on copy-out, DMA."""
                    h = 2 * p + half
                    qt = qg * 4 + j
                    cs = half * 512 + j * P
                    pst = psT.tile([P, S], fp16, tag="pst")
                    for kc in range(KC):
                        nc.tensor.transpose(
                            pst[:, kc * P:(kc + 1) * P],
                            expb_tiles[kc][:, cs:cs + P],
                            idq[:],
                        )
                    eat = eat_pool.tile([P, S], f32, tag="eat")
                    rcol = recip_sb[:, half * 4 + j:half * 4 + j + 1]
                    if on_act:
                        nc.scalar.activation(eat[:], pst[:], Copy, scale=rcol)
                    else:
                        nc.vector.tensor_scalar_mul(eat[:], pst[:], rcol)
                    nc.sync.dma_start(wout.ap()[h, qt * P:(qt + 1) * P, :], eat[:])

                for p in range(CT):          # head pair p -> heads 2p, 2p+1
                    for qg in range(QG):
                        expb_tiles = []
                        for kc in range(KC):
                            emit_b_unit(p, qg, kc, expb_tiles)
                        recip_sb = emit_av_unit(p, qg, expb_tiles)
                        for u in range(8):
                            emit_w_unit(p, qg, u % 4, u // 4, expb_tiles,
                                        recip_sb, on_act=(u % 2 == 1))

            # ---------------- phase 3: output projection ----------------
            psT_cm.__exit__(None, None, None)
            psB_cm.__exit__(None, None, None)
            with (
                tc.tile_pool(name="ph3", bufs=1) as ph3,
                tc.tile_pool(name="psO", bufs=2, space="PSUM") as psO,
            ):
                pout_sb = ph3.tile([P, RT, D], f32, tag="pout")
                for rt in range(RT):
                    for dg in range(2):
                        pso = psO.tile([P, 512], f32, tag="pso")
                        for c in range(CT):
                            nc.tensor.matmul(
                                pso[:],
                                outT_sb[:, c, rt * P:(rt + 1) * P],
                                wot_sb[:, c, dg * 512:(dg + 1) * 512],
                                start=(c == 0), stop=(c == CT - 1),
                            )
                        nc.vector.tensor_copy(
                            pout_sb[:, rt, dg * 512:(dg + 1) * 512], pso[:]
                        )
                nc.sync.dma_start(
                    pout.ap().rearrange("(rt p) d -> p rt d", p=P), pout_sb[:]
                )

    nc.compile()
    return nc


def _get_nc():
    global _NC_CACHE
    if _NC_CACHE is None:
        _NC_CACHE = _build_nc()
    return _NC_CACHE


def _prep_core_inputs(c, x, w_in, b_in, w_out):
    b = c // 2
    hs = (c % 2) * NHC
    ch = slice(hs * HD, hs * HD + CH)

    wq = (w_in[0 * E:1 * E][ch] * SCALING).astype(np.float16)
    wk = w_in[1 * E:2 * E][ch].astype(np.float16)
    wv = w_in[2 * E:3 * E][ch].astype(np.float16)
    bq_ = (b_in[0 * E:1 * E][ch] * SCALING).astype(np.float32)
    bk_ = b_in[1 * E:2 * E][ch].astype(np.float32)
    bv_ = b_in[2 * E:3 * E][ch].astype(np.float32)

    return {
        "xt": np.ascontiguousarray(x[b].T).astype(np.float16),
        "wqt": np.ascontiguousarray(wq.T),
        "wkt": np.ascontiguousarray(wk.T),
        "wvt": np.ascontiguousarray(wv.T),
        "wot": np.ascontiguousarray(w_out[:, ch].T).astype(np.float16),
        "bq": np.ascontiguousarray(bq_.reshape(CH // P, P).T),
        "bk": np.ascontiguousarray(bk_.reshape(CH // P, P).T),
        "bvb": np.ascontiguousarray(
            np.broadcast_to(bv_, (P, CH)).astype(np.float32)),
    }


def run_sharded(x, w_in, b_in, w_out, b_out, trace=False, trace_cores=None):
    """Returns ((output, weights), BassKernelResults)."""
    from concourse import bass_utils

    nc = _get_nc()
    in_maps = [_prep_core_inputs(c, x, w_in, b_in, w_out) for c in range(N_CORES)]
    res = bass_utils.run_bass_kernel_spmd(
        nc, in_maps, core_ids=list(range(N_CORES)),
        trace=trace, trace_cores=trace_cores,
    )
    weights = np.empty((B, H, S, S), np.float32)
    output = np.empty((B, S, D), np.float32)
    for c in range(N_CORES):
        b = c // 2
        hs = (c % 2) * NHC
        weights[b, hs:hs + NHC] = res.results[c]["wout"]
    for b in range(B):
        output[b] = (res.results[2 * b]["pout"] + res.results[2 * b + 1]["pout"]
                     + b_out[None, :])
    return (output, weights), res


def kernel(x, w_in, b_in, w_out, b_out):
    x = np.asarray(x, dtype=np.float32)
    w_in = np.asarray(w_in, dtype=np.float32)
    b_in = np.asarray(b_in, dtype=np.float32)
    w_out = np.asarray(w_out, dtype=np.float32)
    b_out = np.asarray(b_out, dtype=np.float32)
    (output, weights), _ = run_sharded(x, w_in, b_in, w_out, b_out)
    return output, weights


# revision 24
# speedup vs baseline: 1.4227x; 1.0493x over previous
"""Multi-head self-attention (dense_transformer) on 8 Trainium2 NeuronCores.

Problem: B=4, S=2048, D=1024, H=16, E=1024, HD=64.
reference returns (output [B,S,D] f32, weights [B,H,S,S] f32).

Sharding: batch x head-group. Core c handles batch b = c//2 and heads
hs = (c%2)*8 .. hs+8 (8 heads). Per core:
  phase 1: QKV projection from x[b] (fp16 operands, fp32 psum).
           qT,kT stored [head-pair d-dims (128p), S]; V stored [S(k), ch]
           augmented with a ones column per head (softmax sums for free).
  phase 2: scores computed ONLY in [k, q] orientation (PE, head pairs
           row-packed), ACT exp -> fp16 expB tiles.
           attn*V: [V|1]^T @ expB -> psum [d|sum, q]; 1/sum row (DVE) is
           broadcast across partitions by a K=1 ones matmul (PE), then
           DVE multiply -> outT [ch, row] fp16.
           weights output: PE-transposes expB chunks into an fp16 psum
           tile [q, k-block]; per-q 1/sum (tiny PE transposes of the
           recip row) scales it during the psum->SBUF f32 copy
           (alternating DVE / ACT), then DMA.
  phase 3: out-proj partial from outT [ch, row], DMA.
Host: shards/transposes/casts inputs, sums the two partial outputs per batch.
"""

import sys

import numpy as np

if "/opt/trn_rl_repo" not in sys.path:
    sys.path.insert(0, "/opt/trn_rl_repo")

B, S, D = 4, 2048, 1024
H = 16
E = 1024
HD = 64
SCALING = float(HD) ** -0.5

P = 128
NHC = 8          # heads per core
CH = NHC * HD    # 512 channels per core
N_CORES = 8
VW = HD + 1      # v columns per head incl. ones column

_NC_CACHE = None


def _build_nc():
    import concourse.bacc as bacc
    import concourse.mybir as mybir
    import concourse.tile as tile
    from concourse.masks import make_identity

    fp16 = mybir.dt.float16
    f32 = mybir.dt.float32
    Exp = mybir.ActivationFunctionType.Exp
    Copy = mybir.ActivationFunctionType.Copy
    Mult = mybir.AluOpType.mult

    nc = bacc.Bacc("TRN2", target_bir_lowering=False, debug=False)

    xt = nc.dram_tensor("xt", [D, S], fp16, kind="ExternalInput")
    wqt = nc.dram_tensor("wqt", [D, CH], fp16, kind="ExternalInput")
    wkt = nc.dram_tensor("wkt", [D, CH], fp16, kind="ExternalInput")
    wvt = nc.dram_tensor("wvt", [D, CH], fp16, kind="ExternalInput")
    wot = nc.dram_tensor("wot", [CH, D], fp16, kind="ExternalInput")
    bq = nc.dram_tensor("bq", [P, CH // P], f32, kind="ExternalInput")
    bk = nc.dram_tensor("bk", [P, CH // P], f32, kind="ExternalInput")
    bvb = nc.dram_tensor("bvb", [P, CH], f32, kind="ExternalInput")
    wout = nc.dram_tensor("wout", [NHC, S, S], f32, kind="ExternalOutput")
    pout = nc.dram_tensor("pout", [S, D], f32, kind="ExternalOutput")

    FT = D // P          # 8 feature tiles
    RT = S // P          # 16 row tiles
    RG = S // 512        # 4 row/col groups of 512
    CT = CH // P         # 4 chan tiles == head pairs
    KC = S // P          # 16 key chunks
    QG = S // 512        # 4 query groups

    with tile.TileContext(nc) as tc:
        with tc.tile_pool(name="persist", bufs=1) as persist:
            qt_sb = persist.tile([P, CT, S], fp16, tag="qt")
            kt_sb = persist.tile([P, CT, S], fp16, tag="kt")
            vaug_sb = persist.tile([P, RT, NHC * VW], fp16, tag="vaug")
            outT_sb = persist.tile([P, CT, S], fp16, tag="outT")
            wot_sb = persist.tile([P, CT, D], fp16, tag="wot")
            bq_sb = persist.tile([P, CT], f32, tag="bq")
            bk_sb = persist.tile([P, CT], f32, tag="bk")
            bvb_sb = persist.tile([P, CH], f32, tag="bvb")
            ones1 = persist.tile([1, 64], f32, tag="ones1")
            idq = persist.tile([P, P], fp16, tag="idq")

            nc.gpsimd.memset(vaug_sb[:], 1.0)
            nc.gpsimd.memset(ones1[:], 1.0)
            make_identity(nc, idq[:])
            nc.sync.dma_start(wot_sb[:], wot.ap().rearrange("(ct p) d -> p ct d", p=P))
            nc.sync.dma_start(bq_sb[:], bq.ap())
            nc.sync.dma_start(bk_sb[:], bk.ap())
            nc.sync.dma_start(bvb_sb[:], bvb.ap())

            # psum pools shared by all phases:
            # psB: [128,1024] f32 slots x2 (4 banks) - scores / attn*V / recips
            # psT: [128,2048] fp16 slots x2 (4 banks) - transposed weights
            psB_cm = tc.tile_pool(name="psB", bufs=2, space="PSUM")
            psT_cm = tc.tile_pool(name="psT", bufs=2, space="PSUM")
            psB = psB_cm.__enter__()
            psT = psT_cm.__enter__()

            # ---------------- phase 1: QKV ----------------
            with tc.tile_pool(name="ph1", bufs=1) as ph1:
                xt_sb = ph1.tile([P, FT, S], fp16, tag="xt")
                wq_sb = ph1.tile([P, FT, CH], fp16, tag="wq")
                wk_sb = ph1.tile([P, FT, CH], fp16, tag="wk")
                wv_sb = ph1.tile([P, FT, CH], fp16, tag="wv")
                nc.sync.dma_start(xt_sb[:], xt.ap().rearrange("(ft p) r -> p ft r", p=P))
                nc.sync.dma_start(wq_sb[:], wqt.ap().rearrange("(ft p) c -> p ft c", p=P))
                nc.sync.dma_start(wk_sb[:], wkt.ap().rearrange("(ft p) c -> p ft c", p=P))
                nc.sync.dma_start(wv_sb[:], wvt.ap().rearrange("(ft p) c -> p ft c", p=P))

                # q,k in [chan, row] orientation
                for dst, wsb, bias in ((qt_sb, wq_sb, bq_sb), (kt_sb, wk_sb, bk_sb)):
                    for ct in range(CT):
                        for rgp in range(RG // 2):
                            ps = psB.tile([P, 1024], f32, tag="psb")
                            for sub in range(2):
                                rg = 2 * rgp + sub
                                for ft in range(FT):
                                    nc.tensor.matmul(
                                        ps[:, sub * 512:(sub + 1) * 512],
                                        wsb[:, ft, ct * P:(ct + 1) * P],
                                        xt_sb[:, ft, rg * 512:(rg + 1) * 512],
                                        start=(ft == 0), stop=(ft == FT - 1),
                                    )
                            nc.vector.tensor_scalar_add(
                                dst[:, ct, rgp * 1024:(rgp + 1) * 1024], ps[:],
                                bias[:, ct:ct + 1],
                            )
                # v in [row, chan] orientation, interleaved into vaug
                for rtg in range(RT // 2):
                    ps = psT.tile([P, 1024], f32, tag="pst")
                    for sub in range(2):
                        rt = 2 * rtg + sub
                        cs = sub * 512
                        for ft in range(FT):
                            nc.tensor.matmul(
                                ps[:, cs:cs + 512],
                                xt_sb[:, ft, rt * P:(rt + 1) * P],
                                wv_sb[:, ft, :],
                                start=(ft == 0), stop=(ft == FT - 1),
                            )
                        nc.vector.tensor_tensor(
                            vaug_sb[:, rt].rearrange("p (h c) -> p h c", h=NHC)[:, :, :HD],
                            ps[:, cs:cs + 512].rearrange("p (h c) -> p h c", h=NHC),
                            bvb_sb[:].rearrange("p (h c) -> p h c", h=NHC),
                            mybir.AluOpType.add,
                        )

            # ---------------- phase 2: attention ----------------
            with (
                tc.tile_pool(name="eat", bufs=6) as eat_pool,
                tc.tile_pool(name="expb", bufs=32) as expb_pool,
                tc.tile_pool(name="small", bufs=8) as small_pool,
                tc.tile_pool(name="med", bufs=3) as med_pool,
            ):
                def emit_b_unit(p, qg, kc, expb_tiles):
                    psb = psB.tile([P, 1024], f32, tag="psb")
                    for half in (0, 1):
                        po = 64 * half
                        nc.tensor.matmul(
                            psb[:, half * 512:(half + 1) * 512],
                            kt_sb[po:po + 64, p, kc * P:(kc + 1) * P],
                            qt_sb[po:po + 64, p, qg * 512:(qg + 1) * 512],
                            start=True, stop=True,
                        )
                    ebt = expb_pool.tile([P, 1024], fp16, tag="ebt")
                    nc.scalar.activation(ebt[:], psb[:], Exp)
                    expb_tiles.append(ebt)

                def emit_av_unit(p, qg, expb_tiles):
                    # both heads of the pair share one [128, 1024] psum tile
                    # borrowed from the psB pool. Rows 0:64 attn*V, row 64
                    # sums, rows 64:128 reciprocal broadcast (overwrites the
                    # sums row after it has been read).
                    pav = psB.tile([P, 1024], f32, tag="psb")
                    for half in (0, 1):
                        h = 2 * p + half
                        cs = half * 512
                        for kc in range(KC):
                            nc.tensor.matmul(
                                pav[0:VW, cs:cs + 512],
                                vaug_sb[:, kc, h * VW:(h + 1) * VW],
                                expb_tiles[kc][:, cs:cs + 512],
                                start=(kc == 0), stop=(kc == KC - 1),
                            )
                    # Fast path to recip_sb (gates the W-units): copy the sums
                    # rows to SBUF, transpose them to per-q columns on PE, one
                    # small DVE reciprocal.
                    srows = []
                    for half in (0, 1):
                        cs = half * 512
                        srow = med_pool.tile([1, 512], f32, tag="srow")
                        nc.vector.tensor_copy(srow[:], pav[HD:VW, cs:cs + 512])
                        srows.append(srow)
                    recip_ps = psT.tile([P, 8], f32, tag="pst")
                    for half in (0, 1):
                        for j in range(4):
                            nc.tensor.transpose(
                                recip_ps[:, half * 4 + j:half * 4 + j + 1],
                                srows[half][:, j * P:(j + 1) * P],
                                ones1[0:1, 0:1],
                            )
                    recip_sb = small_pool.tile([P, 8], f32, tag="recip_sb")
                    nc.vector.reciprocal(recip_sb[:], recip_ps[:])
                    # Slow tail (off the critical path): normalize attn*V into
                    # outT via reciprocal row -> K=1 broadcast matmul -> mult.
                    for half in (0, 1):
                        po = 64 * half
                        cs = half * 512
                        rrow = med_pool.tile([1, 512], f32, tag="rrow")
                        nc.vector.reciprocal(rrow[:], srows[half][:])
                        nc.tensor.matmul(
                            pav[64:128, cs:cs + 512], ones1[:], rrow[:],
                            start=True, stop=True,
                        )
                        rb_sb = med_pool.tile([64, 512], f32, tag="rb")
                        nc.vector.tensor_copy(rb_sb[:], pav[64:128, cs:cs + 512])
                        nc.vector.tensor_tensor(
                            outT_sb[po:po + 64, p, qg * 512:(qg + 1) * 512],
                            pav[0:HD, cs:cs + 512], rb_sb[:], Mult,
                        )
                    return recip_sb

                def emit_w_unit(p, qg, j, half, expb_tiles, recip_sb, on_act):
                    """weights block (h, qt): transpose 16 expB chunks into an
                    fp16 psum tile, scale by per-q recip on copy-out, DMA."""
                    h = 2 * p + half
                    qt = qg * 4 + j
                    cs = half * 512 + j * P
                    pst = psT.tile([P, S], fp16, tag="pst")
                    for kc in range(KC):
                        nc.tensor.transpose(
                            pst[:, kc * P:(kc + 1) * P],
                            expb_tiles[kc][:, cs:cs + P],
                            idq[:],
                        )
                    eat = eat_pool.tile([P, S], f32, tag="eat")
                    rcol = recip_sb[:, half * 4 + j:half * 4 + j + 1]
                    if on_act:
                        nc.scalar.activation(eat[:], pst[:], Copy, scale=rcol)
                    else:
                        nc.vector.tensor_scalar_mul(eat[:], pst[:], rcol)
                    nc.sync.dma_start(wout.ap()[h, qt * P:(qt + 1) * P, :], eat[:])

                for p in range(CT):          # head pair p -> heads 2p, 2p+1
                    for qg in range(QG):
                        expb_tiles = []
                        for kc in range(KC):
                            emit_b_unit(p, qg, kc, expb_tiles)
                        recip_sb = emit_av_unit(p, qg, expb_tiles)
                        for u in range(8):
                            emit_w_unit(p, qg, u % 4, u // 4, expb_tiles,
                                        recip_sb, on_act=(u % 4 == 3))

            # ---------------- phase 3: output projection ----------------
            psT_cm.__exit__(None, None, None)
            psB_cm.__exit__(None, None, None)
            with (
                tc.tile_pool(name="ph3", bufs=1) as ph3,
                tc.tile_pool(name="psO", bufs=2, space="PSUM") as psO,
            ):
                pout_sb = ph3.tile([P, RT, D], f32, tag="pout")
                for rt in range(RT):
                    for dg in range(2):
                        pso = psO.tile([P, 512], f32, tag="pso")
                        for c in range(CT):
                            nc.tensor.matmul(
                                pso[:],
                                outT_sb[:, c, rt * P:(rt + 1) * P],
                                wot_sb[:, c, dg * 512:(dg + 1) * 512],
                                start=(c == 0), stop=(c == CT - 1),
                            )
                        nc.vector.tensor_copy(
                            pout_sb[:, rt, dg * 512:(dg + 1) * 512], pso[:]
                        )
                nc.sync.dma_start(
                    pout.ap().rearrange("(rt p) d -> p rt d", p=P), pout_sb[:]
                )

    nc.compile()
    return nc


def _get_nc():
    global _NC_CACHE
    if _NC_CACHE is None:
        _NC_CACHE = _build_nc()
    return _NC_CACHE


def _prep_core_inputs(c, x, w_in, b_in, w_out):
    b = c // 2
    hs = (c % 2) * NHC
    ch = slice(hs * HD, hs * HD + CH)

    wq = (w_in[0 * E:1 * E][ch] * SCALING).astype(np.float16)
    wk = w_in[1 * E:2 * E][ch].astype(np.float16)
    wv = w_in[2 * E:3 * E][ch].astype(np.float16)
    bq_ = (b_in[0 * E:1 * E][ch] * SCALING).astype(np.float32)
    bk_ = b_in[1 * E:2 * E][ch].astype(np.float32)
    bv_ = b_in[2 * E:3 * E][ch].astype(np.float32)

    return {
        "xt": np.ascontiguousarray(x[b].T).astype(np.float16),
        "wqt": np.ascontiguousarray(wq.T),
        "wkt": np.ascontiguousarray(wk.T),
        "wvt": np.ascontiguousarray(wv.T),
        "wot": np.ascontiguousarray(w_out[:, ch].T).astype(np.float16),
        "bq": np.ascontiguousarray(bq_.reshape(CH // P, P).T),
        "bk": np.ascontiguousarray(bk_.reshape(CH // P, P).T),
        "bvb": np.ascontiguousarray(
            np.broadcast_to(bv_, (P, CH)).astype(np.float32)),
    }


def run_sharded(x, w_in, b_in, w_out, b_out, trace=False, trace_cores=None):
    """Returns ((output, weights), BassKernelResults)."""
    from concourse import bass_utils

    nc = _get_nc()
    in_maps = [_prep_core_inputs(c, x, w_in, b_in, w_out) for c in range(N_CORES)]
    res = bass_utils.run_bass_kernel_spmd(
        nc, in_maps, core_ids=list(range(N_CORES)),
        trace=trace, trace_cores=trace_cores,
    )
    weights = np.empty((B, H, S, S), np.float32)
    output = np.empty((B, S, D), np.float32)
    for c in range(N_CORES):
        b = c // 2
        hs = (c % 2) * NHC
        weights[b, hs:hs + NHC] = res.results[c]["wout"]
    for b in range(B):
        output[b] = (res.results[2 * b]["pout"] + res.results[2 * b + 1]["pout"]
                     + b_out[None, :])
    return (output, weights), res


def kernel(x, w_in, b_in, w_out, b_out):
    x = np.asarray(x, dtype=np.float32)
    w_in = np.asarray(w_in, dtype=np.float32)
    b_in = np.asarray(b_in, dtype=np.float32)
    w_out = np.asarray(w_out, dtype=np.float32)
    b_out = np.asarray(b_out, dtype=np.float32)
    (output, weights), _ = run_sharded(x, w_in, b_in, w_out, b_out)
    return output, weights


# revision 25
# speedup vs baseline: 1.4805x; 1.0406x over previous
"""Multi-head self-attention (dense_transformer) on 8 Trainium2 NeuronCores.

Problem: B=4, S=2048, D=1024, H=16, E=1024, HD=64.
reference returns (output [B,S,D] f32, weights [B,H,S,S] f32).

Sharding: batch x head-group. Core c handles batch b = c//2 and heads
hs = (c%2)*8 .. hs+8 (8 heads). Per core:
  phase 1: QKV projection from x[b] (fp16 operands, fp32 psum).
           qT,kT stored [head-pair d-dims (128p), S]; V stored [S(k), ch]
           augmented with a ones column per head (softmax sums for free).
  phase 2: scores computed ONLY in [k, q] orientation (PE, head pairs
           row-packed), ACT exp -> fp16 expB tiles.
           attn*V: [V|1]^T @ expB -> psum [d|sum, q]; 1/sum row (DVE) is
           broadcast across partitions by a K=1 ones matmul (PE), then
           DVE multiply -> outT [ch, row] fp16.
           weights output: PE-transposes expB chunks into an fp16 psum
           tile [q, k-block]; per-q 1/sum (tiny PE transposes of the
           recip row) scales it during the psum->SBUF f32 copy
           (alternating DVE / ACT), then DMA.
  phase 3: out-proj partial from outT [ch, row], DMA.
Host: shards/transposes/casts inputs, sums the two partial outputs per batch.
"""

import sys

import numpy as np

if "/opt/trn_rl_repo" not in sys.path:
    sys.path.insert(0, "/opt/trn_rl_repo")

B, S, D = 4, 2048, 1024
H = 16
E = 1024
HD = 64
SCALING = float(HD) ** -0.5

P = 128
NHC = 8          # heads per core
CH = NHC * HD    # 512 channels per core
N_CORES = 8
VW = HD + 1      # v columns per head incl. ones column

_NC_CACHE = None


def _build_nc():
    import concourse.bacc as bacc
    import concourse.mybir as mybir
    import concourse.tile as tile
    from concourse.masks import make_identity

    fp16 = mybir.dt.float16
    f32 = mybir.dt.float32
    Exp = mybir.ActivationFunctionType.Exp
    Copy = mybir.ActivationFunctionType.Copy
    Mult = mybir.AluOpType.mult

    nc = bacc.Bacc("TRN2", target_bir_lowering=False, debug=False)

    xt = nc.dram_tensor("xt", [D, S], fp16, kind="ExternalInput")
    wqt = nc.dram_tensor("wqt", [D, CH], fp16, kind="ExternalInput")
    wkt = nc.dram_tensor("wkt", [D, CH], fp16, kind="ExternalInput")
    wvt = nc.dram_tensor("wvt", [D, CH], fp16, kind="ExternalInput")
    wot = nc.dram_tensor("wot", [CH, D], fp16, kind="ExternalInput")
    bq = nc.dram_tensor("bq", [P, CH // P], f32, kind="ExternalInput")
    bk = nc.dram_tensor("bk", [P, CH // P], f32, kind="ExternalInput")
    bvb = nc.dram_tensor("bvb", [P, CH], f32, kind="ExternalInput")
    wout = nc.dram_tensor("wout", [NHC, S, S], f32, kind="ExternalOutput")
    pout = nc.dram_tensor("pout", [S, D], f32, kind="ExternalOutput")

    FT = D // P          # 8 feature tiles
    RT = S // P          # 16 row tiles
    RG = S // 512        # 4 row/col groups of 512
    CT = CH // P         # 4 chan tiles == head pairs
    KC = S // P          # 16 key chunks
    QG = S // 512        # 4 query groups

    with tile.TileContext(nc) as tc:
        with tc.tile_pool(name="persist", bufs=1) as persist:
            qt_sb = persist.tile([P, CT, S], fp16, tag="qt")
            kt_sb = persist.tile([P, CT, S], fp16, tag="kt")
            vaug_sb = persist.tile([P, RT, NHC * VW], fp16, tag="vaug")
            outT_sb = persist.tile([P, CT, S], fp16, tag="outT")
            wot_sb = persist.tile([P, CT, D], fp16, tag="wot")
            bq_sb = persist.tile([P, CT], f32, tag="bq")
            bk_sb = persist.tile([P, CT], f32, tag="bk")
            bvb_sb = persist.tile([P, CH], f32, tag="bvb")
            ones1 = persist.tile([1, 64], f32, tag="ones1")
            idq = persist.tile([P, P], fp16, tag="idq")

            nc.gpsimd.memset(vaug_sb[:], 1.0)
            nc.gpsimd.memset(ones1[:], 1.0)
            make_identity(nc, idq[:])
            nc.sync.dma_start(wot_sb[:], wot.ap().rearrange("(ct p) d -> p ct d", p=P))
            nc.sync.dma_start(bq_sb[:], bq.ap())
            nc.sync.dma_start(bk_sb[:], bk.ap())
            nc.sync.dma_start(bvb_sb[:], bvb.ap())

            # psum pools shared by all phases:
            # psB: [128,1024] f32 slots x2 (4 banks) - scores / attn*V / recips
            # psT: [128,2048] fp16 slots x2 (4 banks) - transposed weights
            psB_cm = tc.tile_pool(name="psB", bufs=2, space="PSUM")
            psT_cm = tc.tile_pool(name="psT", bufs=2, space="PSUM")
            psB = psB_cm.__enter__()
            psT = psT_cm.__enter__()

            # ---------------- phase 1: QKV ----------------
            with tc.tile_pool(name="ph1", bufs=1) as ph1:
                xt_sb = ph1.tile([P, FT, S], fp16, tag="xt")
                wq_sb = ph1.tile([P, FT, CH], fp16, tag="wq")
                wk_sb = ph1.tile([P, FT, CH], fp16, tag="wk")
                wv_sb = ph1.tile([P, FT, CH], fp16, tag="wv")
                nc.sync.dma_start(xt_sb[:], xt.ap().rearrange("(ft p) r -> p ft r", p=P))
                nc.sync.dma_start(wq_sb[:], wqt.ap().rearrange("(ft p) c -> p ft c", p=P))
                nc.sync.dma_start(wk_sb[:], wkt.ap().rearrange("(ft p) c -> p ft c", p=P))
                nc.sync.dma_start(wv_sb[:], wvt.ap().rearrange("(ft p) c -> p ft c", p=P))

                # q,k in [chan, row] orientation
                for dst, wsb, bias in ((qt_sb, wq_sb, bq_sb), (kt_sb, wk_sb, bk_sb)):
                    for ct in range(CT):
                        for rgp in range(RG // 2):
                            ps = psB.tile([P, 1024], f32, tag="psb")
                            for sub in range(2):
                                rg = 2 * rgp + sub
                                for ft in range(FT):
                                    nc.tensor.matmul(
                                        ps[:, sub * 512:(sub + 1) * 512],
                                        wsb[:, ft, ct * P:(ct + 1) * P],
                                        xt_sb[:, ft, rg * 512:(rg + 1) * 512],
                                        start=(ft == 0), stop=(ft == FT - 1),
                                    )
                            nc.vector.tensor_scalar_add(
                                dst[:, ct, rgp * 1024:(rgp + 1) * 1024], ps[:],
                                bias[:, ct:ct + 1],
                            )
                # v in [row, chan] orientation, interleaved into vaug
                for rtg in range(RT // 2):
                    ps = psT.tile([P, 1024], f32, tag="pst")
                    for sub in range(2):
                        rt = 2 * rtg + sub
                        cs = sub * 512
                        for ft in range(FT):
                            nc.tensor.matmul(
                                ps[:, cs:cs + 512],
                                xt_sb[:, ft, rt * P:(rt + 1) * P],
                                wv_sb[:, ft, :],
                                start=(ft == 0), stop=(ft == FT - 1),
                            )
                        nc.vector.tensor_tensor(
                            vaug_sb[:, rt].rearrange("p (h c) -> p h c", h=NHC)[:, :, :HD],
                            ps[:, cs:cs + 512].rearrange("p (h c) -> p h c", h=NHC),
                            bvb_sb[:].rearrange("p (h c) -> p h c", h=NHC),
                            mybir.AluOpType.add,
                        )

            # ---------------- phase 2: attention ----------------
            with (
                tc.tile_pool(name="eat", bufs=6) as eat_pool,
                tc.tile_pool(name="expb", bufs=32) as expb_pool,
                tc.tile_pool(name="small", bufs=8) as small_pool,
                tc.tile_pool(name="med", bufs=3) as med_pool,
            ):
                def emit_b_unit(p, qg, kc, expb_tiles):
                    psb = psB.tile([P, 1024], f32, tag="psb")
                    for half in (0, 1):
                        po = 64 * half
                        nc.tensor.matmul(
                            psb[:, half * 512:(half + 1) * 512],
                            kt_sb[po:po + 64, p, kc * P:(kc + 1) * P],
                            qt_sb[po:po + 64, p, qg * 512:(qg + 1) * 512],
                            start=True, stop=True,
                        )
                    ebt = expb_pool.tile([P, 1024], fp16, tag="ebt")
                    nc.scalar.activation(ebt[:], psb[:], Exp)
                    expb_tiles.append(ebt)

                def emit_av_unit(p, qg, expb_tiles):
                    # both heads of the pair share one [128, 1024] psum tile
                    # borrowed from the psB pool. Rows 0:64 attn*V, row 64
                    # sums, rows 64:128 reciprocal broadcast (overwrites the
                    # sums row after it has been read).
                    pav = psB.tile([P, 1024], f32, tag="psb")
                    for half in (0, 1):
                        h = 2 * p + half
                        cs = half * 512
                        for kc in range(KC):
                            nc.tensor.matmul(
                                pav[0:VW, cs:cs + 512],
                                vaug_sb[:, kc, h * VW:(h + 1) * VW],
                                expb_tiles[kc][:, cs:cs + 512],
                                start=(kc == 0), stop=(kc == KC - 1),
                            )
                    # Fast path to recip_sb (gates the W-units): copy the sums
                    # rows to SBUF, transpose them to per-q columns on PE, one
                    # small DVE reciprocal.
                    srows = []
                    for half in (0, 1):
                        cs = half * 512
                        srow = med_pool.tile([1, 512], f32, tag="srow")
                        nc.vector.tensor_copy(srow[:], pav[HD:VW, cs:cs + 512])
                        srows.append(srow)
                    recip_ps = psT.tile([P, 8], f32, tag="pst")
                    for half in (0, 1):
                        for j in range(4):
                            nc.tensor.transpose(
                                recip_ps[:, half * 4 + j:half * 4 + j + 1],
                                srows[half][:, j * P:(j + 1) * P],
                                ones1[0:1, 0:1],
                            )
                    recip_sb = small_pool.tile([P, 8], f32, tag="recip_sb")
                    nc.vector.reciprocal(recip_sb[:], recip_ps[:])
                    # Slow tail (off the critical path): normalize attn*V into
                    # outT via reciprocal row -> K=1 broadcast matmul -> mult.
                    for half in (0, 1):
                        po = 64 * half
                        cs = half * 512
                        rrow = med_pool.tile([1, 512], f32, tag="rrow")
                        nc.vector.reciprocal(rrow[:], srows[half][:])
                        nc.tensor.matmul(
                            pav[64:128, cs:cs + 512], ones1[:], rrow[:],
                            start=True, stop=True,
                        )
                        rb_sb = med_pool.tile([64, 512], f32, tag="rb")
                        nc.vector.tensor_copy(rb_sb[:], pav[64:128, cs:cs + 512])
                        nc.vector.tensor_tensor(
                            outT_sb[po:po + 64, p, qg * 512:(qg + 1) * 512],
                            pav[0:HD, cs:cs + 512], rb_sb[:], Mult,
                        )
                    return recip_sb

                def emit_w_unit(p, qg, j, half, expb_tiles, recip_sb, on_act):
                    """weights block (h, qt): transpose 16 expB chunks into an
                    fp16 psum tile, scale by per-q recip on copy-out, DMA."""
                    h = 2 * p + half
                    qt = qg * 4 + j
                    cs = half * 512 + j * P
                    pst = psT.tile([P, S], fp16, tag="pst")
                    for kc in range(KC):
                        nc.tensor.transpose(
                            pst[:, kc * P:(kc + 1) * P],
                            expb_tiles[kc][:, cs:cs + P],
                            idq[:],
                        )
                    eat = eat_pool.tile([P, S], f32, tag="eat")
                    rcol = recip_sb[:, half * 4 + j:half * 4 + j + 1]
                    if on_act:
                        nc.scalar.activation(eat[:], pst[:], Copy, scale=rcol)
                    else:
                        nc.vector.tensor_scalar_mul(eat[:], pst[:], rcol)
                    nc.sync.dma_start(wout.ap()[h, qt * P:(qt + 1) * P, :], eat[:])

                # Two-stage software pipeline over (p, qg) groups: group g's
                # B-units (PE scores + ACT exp) overlap group g-1's attn*V
                # and weights-transpose/copy-out work.
                groups = [(p, qg) for p in range(CT) for qg in range(QG)]
                prev = None
                for p, qg in groups:
                    expb_tiles = []
                    for kc in range(4):
                        emit_b_unit(p, qg, kc, expb_tiles)
                    if prev is not None:
                        pp, pqg, ptiles = prev
                        recip_sb = emit_av_unit(pp, pqg, ptiles)
                    wu = 0
                    for kc in range(4, KC):
                        emit_b_unit(p, qg, kc, expb_tiles)
                        if prev is not None and kc >= 6 and wu < 8:
                            pp, pqg, ptiles = prev
                            emit_w_unit(pp, pqg, wu % 4, wu // 4, ptiles,
                                        recip_sb, on_act=(wu % 4 == 3))
                            wu += 1
                    prev = (p, qg, expb_tiles)
                pp, pqg, ptiles = prev
                recip_sb = emit_av_unit(pp, pqg, ptiles)
                for u in range(8):
                    emit_w_unit(pp, pqg, u % 4, u // 4, ptiles,
                                recip_sb, on_act=(u % 4 == 3))

            # ---------------- phase 3: output projection ----------------
            psT_cm.__exit__(None, None, None)
            psB_cm.__exit__(None, None, None)
            with (
                tc.tile_pool(name="ph3", bufs=1) as ph3,
                tc.tile_pool(name="psO", bufs=2, space="PSUM") as psO,
            ):
                pout_sb = ph3.tile([P, RT, D], f32, tag="pout")
                for rt in range(RT):
                    for dg in range(2):
                        pso = psO.tile([P, 512], f32, tag="pso")
                        for c in range(CT):
                            nc.tensor.matmul(
                                pso[:],
                                outT_sb[:, c, rt * P:(rt + 1) * P],
                                wot_sb[:, c, dg * 512:(dg + 1) * 512],
                                start=(c == 0), stop=(c == CT - 1),
                            )
                        nc.vector.tensor_copy(
                            pout_sb[:, rt, dg * 512:(dg + 1) * 512], pso[:]
                        )
                nc.sync.dma_start(
                    pout.ap().rearrange("(rt p) d -> p rt d", p=P), pout_sb[:]
                )

    nc.compile()
    return nc


def _get_nc():
    global _NC_CACHE
    if _NC_CACHE is None:
        _NC_CACHE = _build_nc()
    return _NC_CACHE


def _prep_core_inputs(c, x, w_in, b_in, w_out):
    b = c // 2
    hs = (c % 2) * NHC
    ch = slice(hs * HD, hs * HD + CH)

    wq = (w_in[0 * E:1 * E][ch] * SCALING).astype(np.float16)
    wk = w_in[1 * E:2 * E][ch].astype(np.float16)
    wv = w_in[2 * E:3 * E][ch].astype(np.float16)
    bq_ = (b_in[0 * E:1 * E][ch] * SCALING).astype(np.float32)
    bk_ = b_in[1 * E:2 * E][ch].astype(np.float32)
    bv_ = b_in[2 * E:3 * E][ch].astype(np.float32)

    return {
        "xt": np.ascontiguousarray(x[b].T).astype(np.float16),
        "wqt": np.ascontiguousarray(wq.T),
        "wkt": np.ascontiguousarray(wk.T),
        "wvt": np.ascontiguousarray(wv.T),
        "wot": np.ascontiguousarray(w_out[:, ch].T).astype(np.float16),
        "bq": np.ascontiguousarray(bq_.reshape(CH // P, P).T),
        "bk": np.ascontiguousarray(bk_.reshape(CH // P, P).T),
        "bvb": np.ascontiguousarray(
            np.broadcast_to(bv_, (P, CH)).astype(np.float32)),
    }


def run_sharded(x, w_in, b_in, w_out, b_out, trace=False, trace_cores=None):
    """Returns ((output, weights), BassKernelResults)."""
    from concourse import bass_utils

    nc = _get_nc()
    in_maps = [_prep_core_inputs(c, x, w_in, b_in, w_out) for c in range(N_CORES)]
    res = bass_utils.run_bass_kernel_spmd(
        nc, in_maps, core_ids=list(range(N_CORES)),
        trace=trace, trace_cores=trace_cores,
    )
    weights = np.empty((B, H, S, S), np.float32)
    output = np.empty((B, S, D), np.float32)
    for c in range(N_CORES):
        b = c // 2
        hs = (c % 2) * NHC
        weights[b, hs:hs + NHC] = res.results[c]["wout"]
    for b in range(B):
        output[b] = (res.results[2 * b]["pout"] + res.results[2 * b + 1]["pout"]
                     + b_out[None, :])
    return (output, weights), res


def kernel(x, w_in, b_in, w_out, b_out):
    x = np.asarray(x, dtype=np.float32)
    w_in = np.asarray(w_in, dtype=np.float32)
    b_in = np.asarray(b_in, dtype=np.float32)
    w_out = np.asarray(w_out, dtype=np.float32)
    b_out = np.asarray(b_out, dtype=np.float32)
    (output, weights), _ = run_sharded(x, w_in, b_in, w_out, b_out)
    return output, weights
